# revision 1
# baseline (speedup 1.0000x reference)
"""GAT edge classifier on 8 Trainium2 NeuronCores.

Strategy: edges sorted by destination on host; destination nodes are
partitioned contiguously across the 8 cores (1250 each), so each core owns
all edges of its dst range and the segment softmax needs no cross-core
reduction. Gather tables (node features + attention terms, packed rows) are
built shard-wise and AllGathered between layers. Per-edge aggregation is
done with per-chunk incidence-matrix matmuls (segment-sum as dense matmul
over 128-edge chunks); feature gathers via dma_gather.

Self-loops (PyG add_self_loops with fill_value='mean') are appended as one
dedicated 128-slot chunk per 128-node block; their edge_attr contribution
(mean of incoming edge attrs) is computed on device in a pre-pass.

The segment-softmax max-subtraction is skipped: exp(a-m)/sum exp(a-m) ==
exp(a)/sum exp(a) exactly, and alpha is O(1) here so f32 exp is safe.
"""
import sys
if "/opt/trn_rl_repo" not in sys.path:
    sys.path.insert(0, "/opt/trn_rl_repo")

import numpy as np
import ml_dtypes

NCORES = 8
N = 10000
H = 8
C = 64
HC = H * C          # 512
EDIM = 3
ENC = 128
NPC = N // NCORES   # 1250 nodes per core
NB = (NPC + 127) // 128  # 10 blocks (last has 98 real nodes)
ECPB = 36           # edge chunks per block (capacity 36*128=4608 real edges)
CPB = ECPB + 1      # + 1 self-loop chunk
CH = NB * CPB       # 370 chunks per core
SPB = CPB * 128     # 4736 slots per block
S = NB * SPB        # 47360 slots per core

USE_BF16 = True
MOCK_CC = False


# ---------------------------------------------------------------- program --

def _build_program(use_bf16, stages=99, sub=99):
    import concourse.bacc as bacc
    import concourse.mybir as mybir
    import concourse.tile as tile
    from concourse import library_config

    f32 = mybir.dt.float32
    bf16 = mybir.dt.bfloat16
    i16 = mybir.dt.int16
    i32 = mybir.dt.int32
    OP = mybir.AluOpType
    AFT = mybir.ActivationFunctionType

    TDT = bf16 if use_bf16 else f32
    R1 = 640 if use_bf16 else 576     # T1 row elems ([xl(512)|asrc(8)|pad])
    RD = 128 if use_bf16 else 64      # Td row elems ([adst(8)|pad]) == 256B
    RS = 128                          # T0/T2/Tm row elems ([feat|asrc|pad])
    GSPL = [10, 9, 9, 9] if use_bf16 else [7, 6, 6, 6, 6, 6]  # T1-gather split

    nc = bacc.Bacc("TRN2", target_bir_lowering=False, debug=False,
                   num_devices=NCORES)

    def ein(name, shape, dt):
        return nc.dram_tensor(name, shape, dt, kind="ExternalInput")

    # ---- external inputs (per core) ----
    xT = ein("xT", [2, NB * 128], f32)
    srcidx = ein("srcidx", [128, S // 16], i16)
    tdidx = ein("tdidx", [128, S // 16], i16)
    mdidx = ein("mdidx", [128, S // 16], i16)
    dstloc = ein("dstloc", [128, CH], i16)
    prevals = ein("prevals", [128, CH * 4], TDT)
    eaT = ein("eaT", [3, S], f32)
    m3 = ein("m3", [3, 24], f32)
    m3rep = ein("m3rep", [128, 3 * 24], f32)
    encw1 = ein("encw1", [2, ENC], f32)
    encb1 = ein("encb1", [ENC, 1], f32)
    encw2 = ein("encw2", [ENC, C], TDT)
    encb2 = ein("encb2", [C, 1], f32)
    waug0s = ein("waug0s", [C, 16], TDT)
    w0 = ein("w0", [C, HC], TDT)
    waug1 = ein("waug1", [4, 128, HC], TDT)
    waug1s = ein("waug1s", [4, 128, 16], TDT)
    waug2 = ein("waug2", [4, 128, C + 2], TDT)
    b0r = ein("b0r", [128, HC], f32)
    b1r = ein("b1r", [128, HC], f32)
    b2r = ein("b2r", [128, C], f32)
    mw1s = ein("mw1s", [C, C], bf16)
    mw1d = ein("mw1d", [C, C], bf16)
    mw2 = ein("mw2", [C, 3], bf16)
    mb1 = ein("mb1", [C, 1], f32)
    mb2 = ein("mb2", [3, 1], f32)
    iotaf = ein("iotaf", [128, 128], i16)
    identf = ein("identf", [128, 128], TDT)

    out = nc.dram_tensor("out", [3, S], f32, kind="ExternalOutput")

    # ---- internal DRAM ----
    def idram(name, shape, dt, shared=False):
        return nc.dram_tensor(name, shape, dt, kind="Internal",
                              addr_space="Shared" if shared else "Local")

    T0s = idram("T0s", [NPC, RS], TDT)
    T0f = idram("T0f", [N, RS], TDT, shared=True)
    T1s = idram("T1s", [NPC, R1], TDT)
    T1f = idram("T1f", [N, R1], TDT, shared=True)
    T2s = idram("T2s", [NPC, RS], TDT)
    T2f = idram("T2f", [N, RS], TDT, shared=True)
    Tms = idram("Tms", [NPC, RS], bf16)
    Tmf = idram("Tmf", [N, RS], bf16, shared=True)
    Td0 = idram("Td0", [NB * 128, RD], TDT)
    Td1 = idram("Td1", [NB * 128, RD], TDT)
    Td2 = idram("Td2", [NB * 128, RD], TDT)

    RG = [list(range(NCORES))]

    with tile.TileContext(nc) as tc:
      with tc.tile_pool(name="cst", bufs=1) as cst:

        # ---------------- constants / small loads ----------------
        nc.gpsimd.load_library(library_config.mlp)
        iota_f = cst.tile([128, 128], i16, tag="iota_f")
        nc.sync.dma_start(iota_f[:], iotaf[:])
        ident = cst.tile([128, 128], TDT, tag="ident")
        nc.sync.dma_start(ident[:], identf[:])

        dl_t = cst.tile([128, CH], i16, tag="dl")
        nc.sync.dma_start(dl_t[:], dstloc[:])
        pv_t = cst.tile([128, CH, 4], TDT, tag="pv")
        nc.sync.dma_start(pv_t[:], prevals[:].rearrange("p (c v) -> p c v", v=4))
        si_t = cst.tile([128, S // 16], i16, tag="si")
        nc.sync.dma_start(si_t[:], srcidx[:])
        ti_t = cst.tile([128, S // 16], i16, tag="ti")
        nc.sync.dma_start(ti_t[:], tdidx[:])
        mi_t = cst.tile([128, S // 16], i16, tag="mi")
        nc.sync.dma_start(mi_t[:], mdidx[:])
        m3_t = cst.tile([3, 24], f32, tag="m3")
        nc.sync.dma_start(m3_t[:], m3[:])
        m3r_t = cst.tile([128, 3, 24], f32, tag="m3r")
        nc.sync.dma_start(m3r_t[:], m3rep[:].rearrange("p (a b) -> p a b", a=3))
        xT_t = cst.tile([2, NB * 128], f32, tag="xT")
        nc.sync.dma_start(xT_t[:], xT[:])
        ew1_t = cst.tile([2, ENC], f32, tag="ew1")
        nc.sync.dma_start(ew1_t[:], encw1[:])
        eb1_t = cst.tile([ENC, 1], f32, tag="eb1")
        nc.sync.dma_start(eb1_t[:], encb1[:])
        ew2_t = cst.tile([ENC, C], TDT, tag="ew2")
        nc.sync.dma_start(ew2_t[:], encw2[:])
        eb2_t = cst.tile([C, 1], f32, tag="eb2")
        nc.sync.dma_start(eb2_t[:], encb2[:])
        wa0s_t = cst.tile([C, 16], TDT, tag="wa0s")
        nc.sync.dma_start(wa0s_t[:], waug0s[:])
        w0_t = cst.tile([C, HC], TDT, tag="w0")
        nc.sync.dma_start(w0_t[:], w0[:])
        b0_t = cst.tile([128, HC], f32, tag="b0")
        nc.sync.dma_start(b0_t[:], b0r[:])
        b1_t = cst.tile([128, HC], f32, tag="b1")
        nc.sync.dma_start(b1_t[:], b1r[:])
        b2_t = cst.tile([128, C], f32, tag="b2")
        nc.sync.dma_start(b2_t[:], b2r[:])
        wa1_t = cst.tile([128, 4, HC], TDT, tag="wa1")
        nc.sync.dma_start(wa1_t[:], waug1[:].transpose([1, 0, 2]))
        wa1s_t = cst.tile([128, 4, 16], TDT, tag="wa1s")
        nc.sync.dma_start(wa1s_t[:], waug1s[:].transpose([1, 0, 2]))
        wa2_t = cst.tile([128, 4, C + 2], TDT, tag="wa2")
        nc.sync.dma_start(wa2_t[:], waug2[:].transpose([1, 0, 2]))
        mw1s_t = cst.tile([C, C], bf16, tag="mw1s")
        nc.sync.dma_start(mw1s_t[:], mw1s[:])
        mw1d_t = cst.tile([C, C], bf16, tag="mw1d")
        nc.sync.dma_start(mw1d_t[:], mw1d[:])
        mw2_t = cst.tile([C, 3], bf16, tag="mw2")
        nc.sync.dma_start(mw2_t[:], mw2[:])
        mb1_t = cst.tile([C, 1], f32, tag="mb1")
        nc.sync.dma_start(mb1_t[:], mb1[:])
        mb2_t = cst.tile([3, 1], f32, tag="mb2")
        nc.sync.dma_start(mb2_t[:], mb2[:])

        mean_t = cst.tile([128, NB, 3], f32, tag="mean")
        ae3_t = cst.tile([128, CH, 24], f32, tag="ae3")

        # ---------------- stage 1: encoder (own nodes) ----------------
        h0T_t = cst.tile([C, NB * 128], TDT, tag="h0T")
        with tc.tile_pool(name="encp", bufs=2, space="PSUM") as encp, \
             tc.tile_pool(name="encs", bufs=2) as encs:
            for sl0 in range(0, NB * 128, 512):
                w = min(512, NB * 128 - sl0)
                p1 = encp.tile([ENC, 512], f32, tag="p1")
                nc.tensor.matmul(p1[:, :w], lhsT=ew1_t[:],
                                 rhs=xT_t[:, sl0:sl0 + w],
                                 start=True, stop=True)
                r1 = encs.tile([ENC, 512], TDT, tag="r1")
                nc.scalar.activation(r1[:, :w], p1[:, :w], AFT.Relu,
                                     bias=eb1_t[:])
                p2 = encp.tile([C, 512], f32, tag="p2")
                nc.tensor.matmul(p2[:, :w], lhsT=ew2_t[:], rhs=r1[:, :w],
                                 start=True, stop=True)
                nc.vector.tensor_scalar(out=h0T_t[:, sl0:sl0 + w],
                                        in0=p2[:, :w], scalar1=eb2_t[:],
                                        scalar2=None, op0=OP.add)

        # ---------------- stage 2: T0/Td0 build + AG0 ----------------
        with tc.tile_pool(name="t0p", bufs=2, space="PSUM") as t0p, \
             tc.tile_pool(name="t0s", bufs=2) as t0s:
            for b in range(NB):
                lhs = h0T_t[:, b * 128:(b + 1) * 128]
                pa = t0p.tile([128, 16], f32, tag="pa")
                nc.tensor.matmul(pa[:], lhsT=lhs, rhs=wa0s_t[:],
                                 start=True, stop=True)
                ptr = t0p.tile([128, C], TDT, tag="ptr")
                nc.tensor.transpose(ptr[:], lhs, ident[0:C, 0:C])
                t0 = t0s.tile([128, RS], TDT, tag="t0")
                nc.vector.tensor_copy(t0[:, 0:C], ptr[:])
                nc.vector.tensor_copy(t0[:, C:C + 8], pa[:, 0:8])
                td = t0s.tile([128, RD], TDT, tag="td")
                nc.vector.tensor_copy(td[:, 0:8], pa[:, 8:16])
                rows = min(128, NPC - b * 128)
                nc.sync.dma_start(T0s[b * 128:b * 128 + rows, :], t0[:rows, :])
                nc.sync.dma_start(Td0[b * 128:(b + 1) * 128, :], td[:])
        if MOCK_CC:
            nc.sync.dma_start(T0f[0:NPC, :], T0s[:, :])
        else:
            nc.gpsimd.collective_compute(
                "AllGather", OP.bypass, replica_groups=RG,
                ins=[T0s[:, :]], outs=[T0f[:, :]])

        if stages < 3:
            return nc

        # ---------------- stage 4: ae_all3 = ea @ [M0|M1|M2] ----------------
        if stages < 4:
            return nc
        with tc.tile_pool(name="aep", bufs=2, space="PSUM") as aep, \
             tc.tile_pool(name="aes", bufs=3) as aes:
            for b in range(NB):
                ea_b = aes.tile([3, SPB], f32, tag="ea_b")
                nc.sync.dma_start(ea_b[:], eaT[:, b * SPB:(b + 1) * SPB])
                for c in range(CPB):
                    ci = b * CPB + c
                    pae = aep.tile([128, 24], f32, tag="pae")
                    nc.tensor.matmul(pae[:],
                                     lhsT=ea_b[:, c * 128:(c + 1) * 128],
                                     rhs=m3_t[:], start=True, stop=True)
                    nc.vector.tensor_copy(ae3_t[:, ci, :], pae[:])

        # ---- next-table builders (called per block from gat_layer) ----
        def build_T1(b, hn, ls, lp):
            pxl = lp.tile([128, HC], f32, tag="outp")
            pxs = lp.tile([128, 16], f32, tag="pxs")
            for kc in range(4):
                ptr = lp.tile([128, 128], TDT, tag="ptrT")
                nc.tensor.transpose(ptr[:], hn[:, kc * 128:(kc + 1) * 128],
                                    ident[:])
                hT = ls.tile([128, 128], TDT, tag="hT")
                nc.vector.tensor_copy(hT[:], ptr[:])
                nc.tensor.matmul(pxl[:], lhsT=hT[:], rhs=wa1_t[:, kc, :],
                                 start=(kc == 0), stop=(kc == 3))
                nc.tensor.matmul(pxs[:], lhsT=hT[:], rhs=wa1s_t[:, kc, :],
                                 start=(kc == 0), stop=(kc == 3))
            t1 = ls.tile([128, R1], TDT, tag="t1")
            nc.vector.tensor_copy(t1[:, 0:HC], pxl[:])
            nc.vector.tensor_copy(t1[:, HC:HC + 8], pxs[:, 0:8])
            td = ls.tile([128, RD], TDT, tag="td1")
            nc.vector.tensor_copy(td[:, 0:8], pxs[:, 8:16])
            rows = min(128, NPC - b * 128)
            nc.sync.dma_start(T1s[b * 128:b * 128 + rows, :], t1[:rows, :])
            nc.sync.dma_start(Td1[b * 128:(b + 1) * 128, :], td[:])

        def build_T2(b, hn, ls, lp):
            pxl = lp.tile([128, C + 2], f32, tag="outp")
            for kc in range(4):
                ptr = lp.tile([128, 128], TDT, tag="ptrT")
                nc.tensor.transpose(ptr[:], hn[:, kc * 128:(kc + 1) * 128],
                                    ident[:])
                hT = ls.tile([128, 128], TDT, tag="hT")
                nc.vector.tensor_copy(hT[:], ptr[:])
                nc.tensor.matmul(pxl[:], lhsT=hT[:], rhs=wa2_t[:, kc, :],
                                 start=(kc == 0), stop=(kc == 3))
            t2 = ls.tile([128, RS], TDT, tag="t1")
            nc.vector.tensor_copy(t2[:, 0:C + 1], pxl[:, 0:C + 1])
            td = ls.tile([128, RD], TDT, tag="td1")
            nc.vector.tensor_copy(td[:, 0:1], pxl[:, C + 1:C + 2])
            rows = min(128, NPC - b * 128)
            nc.sync.dma_start(T2s[b * 128:b * 128 + rows, :], t2[:rows, :])
            nc.sync.dma_start(Td2[b * 128:(b + 1) * 128, :], td[:])

        def build_Tm(b, hn, ls, lp):
            tm = ls.tile([128, RS], bf16, tag="t1")
            nc.vector.tensor_copy(tm[:, 0:C], hn[:, 0:C])
            rows = min(128, NPC - b * 128)
            nc.sync.dma_start(Tms[b * 128:b * 128 + rows, :], tm[:rows, :])

        # ---------------- GAT layer ----------------
        def gat_layer(lidx, Tf, Td, row_elems, asrc_col, hd, bias_t,
                      build_next, is_l0, gspl):
            fwid = HC if (is_l0 or hd == H) else C   # agg/feature width
            with tc.tile_pool(name=f"l{lidx}p", bufs=2, space="PSUM") as lp, \
                 tc.tile_pool(name=f"l{lidx}q", bufs=1, space="PSUM") as lq, \
                 tc.tile_pool(name=f"l{lidx}s", bufs=3) as ls, \
                 tc.tile_pool(name=f"l{lidx}g", bufs=3) as lg:
                for b in range(NB):
                    c0 = b * CPB
                    # gathers (T-gather split for SBUF footprint)
                    gts = []
                    cacc = 0
                    for cn in gspl:
                        g = lg.tile([128, gspl[0], row_elems], TDT, tag="g")
                        nidx = cn * 128
                        off16 = (b * SPB + cacc * 128) // 16
                        nc.gpsimd.dma_gather(
                            out_ap=g[:, :cn, :], in_ap=Tf[:, :],
                            idxs_ap=si_t[:, off16:off16 + nidx // 16],
                            num_idxs=nidx, num_idxs_reg=nidx,
                            elem_size=row_elems, single_packet=False)
                        gts.append((g, cacc, cn))
                        cacc += cn
                    gd = lg.tile([128, CPB, RD], TDT, tag="gd")
                    nc.gpsimd.dma_gather(
                        out_ap=gd[:], in_ap=Td[:, :],
                        idxs_ap=ti_t[:, b * SPB // 16:(b + 1) * SPB // 16],
                        num_idxs=SPB, num_idxs_reg=SPB, elem_size=RD,
                        single_packet=False)

                    if sub < 2:
                        continue
                    # batched A build for the block
                    Ab = ls.tile([128, CPB, 128], TDT, tag="Abat")
                    nc.vector.tensor_tensor(
                        out=Ab[:],
                        in0=dl_t[:, c0:c0 + CPB, None].to_broadcast(
                            [128, CPB, 128]),
                        in1=iota_f[:, None, :].to_broadcast([128, CPB, 128]),
                        op=OP.is_equal)

                    if is_l0:
                        # pre-pass (deg + mean edge_attr) reusing this
                        # block's A, then self-loop ae corrections for all
                        # three layers (written before the self-loop chunk's
                        # alpha assembly reads ae3_t).
                        pre = lq.tile([128, 4], f32, tag="pre")
                        for c in range(CPB):
                            nc.tensor.matmul(pre[:], lhsT=Ab[:, c, :],
                                             rhs=pv_t[:, c0 + c, :],
                                             start=(c == 0),
                                             stop=(c == CPB - 1))
                        deg = ls.tile([128, 1], f32, tag="deg")
                        nc.vector.tensor_scalar(out=deg[:], in0=pre[:, 3:4],
                                                scalar1=1.0, scalar2=None,
                                                op0=OP.max)
                        nc.vector.reciprocal(deg[:], deg[:])
                        nc.vector.tensor_scalar(out=mean_t[:, b, :],
                                                in0=pre[:, 0:3],
                                                scalar1=deg[:], scalar2=None,
                                                op0=OP.mult)
                        ci = c0 + CPB - 1
                        acc = ls.tile([128, 24], f32, tag="acc")
                        nc.vector.tensor_tensor(
                            out=acc[:],
                            in0=mean_t[:, b, 0:1].to_broadcast([128, 24]),
                            in1=m3r_t[:, 0, :], op=OP.mult)
                        for d in (1, 2):
                            t2c = ls.tile([128, 24], f32, tag="t2c")
                            nc.vector.tensor_tensor(
                                out=t2c[:],
                                in0=mean_t[:, b, d:d + 1].to_broadcast(
                                    [128, 24]),
                                in1=m3r_t[:, d, :], op=OP.mult)
                            nc.vector.tensor_tensor(out=acc[:], in0=acc[:],
                                                    in1=t2c[:], op=OP.add)
                        nc.vector.tensor_copy(ae3_t[:, ci, :], acc[:])

                    # alpha assembly + exp (per gather split, so each
                    # gather tile's lifetime closes before later splits
                    # need pool slots)
                    al = ls.tile([128, CPB, hd], f32, tag="al")
                    ext = ls.tile([128, CPB, hd], TDT, tag="ext")
                    for g, ca, cn in gts:
                        sl = al[:, ca:ca + cn, :]
                        nc.vector.tensor_tensor(
                            out=sl,
                            in0=g[:, :cn, asrc_col:asrc_col + hd],
                            in1=gd[:, ca:ca + cn, 0:hd], op=OP.add)
                        nc.vector.tensor_tensor(
                            out=sl, in0=sl,
                            in1=ae3_t[:, c0 + ca:c0 + ca + cn,
                                      lidx * 8:lidx * 8 + hd],
                            op=OP.add)
                        nc.vector.scalar_tensor_tensor(
                            out=sl, in0=sl, scalar=0.2, in1=sl,
                            op0=OP.mult, op1=OP.max)
                        nc.scalar.activation(ext[:, ca:ca + cn, :], sl,
                                             AFT.Exp)

                    if sub < 3:
                        continue
                    # chunk loop: aggregation matmuls
                    agg = lp.tile([128, fwid], f32, tag="agg")
                    den = lp.tile([128, hd], f32, tag="den")
                    for c in range(CPB):
                        g, ca, cn = None, 0, 0
                        for gg, cca, ccn in gts:
                            if cca <= c < cca + ccn:
                                g, ca, cn = gg, cca, ccn
                                break
                        vals = ls.tile([128, fwid], TDT, tag="vals")
                        if is_l0:
                            nc.vector.tensor_tensor(
                                out=vals[:].rearrange("p (a b) -> p a b", a=hd),
                                in0=g[:, c - ca, None, 0:C].to_broadcast(
                                    [128, hd, C]),
                                in1=ext[:, c, :, None].to_broadcast(
                                    [128, hd, C]),
                                op=OP.mult)
                        elif hd == 1:
                            nc.vector.tensor_tensor(
                                out=vals[:],
                                in0=g[:, c - ca, 0:C],
                                in1=ext[:, c, 0:1].to_broadcast([128, C]),
                                op=OP.mult)
                        else:
                            nc.vector.tensor_tensor(
                                out=vals[:].rearrange("p (a b) -> p a b", a=hd),
                                in0=g[:, c - ca, 0:HC].rearrange(
                                    "p (a b) -> p a b", a=hd),
                                in1=ext[:, c, :, None].to_broadcast(
                                    [128, hd, C]),
                                op=OP.mult)
                        nc.tensor.matmul(agg[:], lhsT=Ab[:, c, :], rhs=vals[:],
                                         start=(c == 0), stop=(c == CPB - 1))
                        nc.tensor.matmul(den[:], lhsT=Ab[:, c, :],
                                         rhs=ext[:, c, :],
                                         start=(c == 0), stop=(c == CPB - 1))

                    if sub < 4:
                        continue
                    # finalize: divide, (L0: per-head W0 matmul), bias, elu
                    rec = ls.tile([128, hd], f32, tag="recd")
                    nc.vector.tensor_scalar(out=rec[:], in0=den[:],
                                            scalar1=1e-16, scalar2=None,
                                            op0=OP.add)
                    nc.vector.reciprocal(rec[:], rec[:])
                    aggs = ls.tile([128, fwid], TDT, tag="aggs")
                    if hd == 1:
                        nc.vector.tensor_tensor(
                            out=aggs[:], in0=agg[:],
                            in1=rec[:, 0:1].to_broadcast([128, C]),
                            op=OP.mult)
                    else:
                        nc.vector.tensor_tensor(
                            out=aggs[:].rearrange("p (a b) -> p a b", a=hd),
                            in0=agg[:].rearrange("p (a b) -> p a b", a=hd),
                            in1=rec[:, :, None].to_broadcast([128, hd, C]),
                            op=OP.mult)
                    if is_l0:
                        outp = lq.tile([128, HC], f32, tag="outp")
                        for h in range(H):
                            ptr = lq.tile([C, 128], TDT, tag="ptrT")
                            nc.tensor.transpose(
                                ptr[:], aggs[:, h * C:(h + 1) * C], ident[:])
                            aT = ls.tile([C, 128], TDT, tag="aT")
                            nc.vector.tensor_copy(aT[:], ptr[:])
                            nc.tensor.matmul(
                                outp[:, h * C:(h + 1) * C], lhsT=aT[:],
                                rhs=w0_t[:, h * C:(h + 1) * C],
                                start=True, stop=True)
                        pre_act = outp[:]
                        fc = HC
                    else:
                        pre_act = agg[:, 0:fwid]
                        fc = fwid
                        if hd == H:
                            pre_act = aggs  # placeholder; replaced below
                    # note: for hd==8 non-l0, division already applied (aggs);
                    # for l0 path, aggs was divided before W0. For hd==1,
                    # aggs divided too. So always use aggs except l0 (outp).
                    if is_l0:
                        src_t = outp[:]
                    else:
                        src_t = aggs[:]
                    hb = ls.tile([128, fc], f32, tag="hb")
                    nc.vector.tensor_tensor(out=hb[:], in0=src_t,
                                            in1=bias_t[:, 0:fc], op=OP.add)
                    # elu: relu(x) + exp(min(x,0)) - 1
                    tmin = ls.tile([128, fc], f32, tag="tmin")
                    nc.vector.tensor_scalar(out=tmin[:], in0=hb[:],
                                            scalar1=0.0, scalar2=None,
                                            op0=OP.min)
                    ee = ls.tile([128, fc], f32, tag="ee")
                    nc.scalar.activation(ee[:], tmin[:], AFT.Exp)
                    nc.vector.tensor_scalar(out=ee[:], in0=ee[:],
                                            scalar1=-1.0, scalar2=None,
                                            op0=OP.add)
                    hn = ls.tile([128, fc], TDT, tag="hn")
                    nc.vector.scalar_tensor_tensor(
                        out=hn[:], in0=hb[:], scalar=0.0, in1=ee[:],
                        op0=OP.max, op1=OP.add)
                    if sub < 5:
                        continue
                    build_next(b, hn, ls, lq)

        # ---------------- layers + collectives ----------------
        if stages < 5:
            return nc
        gat_layer(0, T0f, Td0, RS, C, H, b0_t, build_T1, True, [10, 9, 9, 9])
        if MOCK_CC:
            nc.sync.dma_start(T1f[0:NPC, :], T1s[:, :])
        else:
            nc.gpsimd.collective_compute(
                "AllGather", OP.bypass, replica_groups=RG,
                ins=[T1s[:, :]], outs=[T1f[:, :]])

        if stages < 6:
            return nc
        gat_layer(1, T1f, Td1, R1, HC, H, b1_t, build_T2, False, GSPL)
        if MOCK_CC:
            nc.sync.dma_start(T2f[0:NPC, :], T2s[:, :])
        else:
            nc.gpsimd.collective_compute(
                "AllGather", OP.bypass, replica_groups=RG,
                ins=[T2s[:, :]], outs=[T2f[:, :]])

        if stages < 7:
            return nc
        gat_layer(2, T2f, Td2, RS, C, 1, b2_t, build_Tm, False, [10, 9, 9, 9])
        if MOCK_CC:
            nc.sync.dma_start(Tmf[0:NPC, :], Tms[:, :])
        else:
            nc.gpsimd.collective_compute(
                "AllGather", OP.bypass, replica_groups=RG,
                ins=[Tms[:, :]], outs=[Tmf[:, :]])

        # ---------------- edge MLP ----------------
        if stages < 8:
            return nc
        with tc.tile_pool(name="mlpp", bufs=2, space="PSUM") as mp, \
             tc.tile_pool(name="mlps", bufs=3) as ms, \
             tc.tile_pool(name="mlpg", bufs=3) as mg:
            for b in range(NB):
                zs = mg.tile([128, 1, SPB], bf16, tag="zs")
                nc.gpsimd.dma_gather(
                    out_ap=zs[:], in_ap=Tmf[:, :],
                    idxs_ap=si_t[:, b * SPB // 16:(b + 1) * SPB // 16],
                    num_idxs=SPB, num_idxs_reg=SPB, elem_size=RS,
                    transpose=True, single_packet=False)
                zd = mg.tile([128, 1, SPB], bf16, tag="zd")
                nc.gpsimd.dma_gather(
                    out_ap=zd[:], in_ap=Tmf[:, :],
                    idxs_ap=mi_t[:, b * SPB // 16:(b + 1) * SPB // 16],
                    num_idxs=SPB, num_idxs_reg=SPB, elem_size=RS,
                    transpose=True, single_packet=False)
                ob = ms.tile([3, SPB], f32, tag="ob")
                for g0 in range(0, SPB, 512):
                    w = min(512, SPB - g0)
                    pr1 = mp.tile([C, 512], f32, tag="pr1")
                    nc.tensor.matmul(pr1[:, :w], lhsT=mw1s_t[:],
                                     rhs=zs[0:C, 0, g0:g0 + w],
                                     start=True, stop=False)
                    nc.tensor.matmul(pr1[:, :w], lhsT=mw1d_t[:],
                                     rhs=zd[0:C, 0, g0:g0 + w],
                                     start=False, stop=True)
                    r1 = ms.tile([C, 512], bf16, tag="r1m")
                    nc.vector.tensor_scalar(out=r1[:, :w], in0=pr1[:, :w],
                                            scalar1=mb1_t[:], scalar2=0.0,
                                            op0=OP.add, op1=OP.max)
                    po = mp.tile([3, 512], f32, tag="po")
                    nc.tensor.matmul(po[:, :w], lhsT=mw2_t[:], rhs=r1[:, :w],
                                     start=True, stop=True)
                    nc.vector.tensor_scalar(out=ob[:, g0:g0 + w],
                                            in0=po[:, :w], scalar1=mb2_t[:],
                                            scalar2=None, op0=OP.add)
                nc.sync.dma_start(out[:, b * SPB:(b + 1) * SPB], ob[:])

    return nc


# ---------------------------------------------------------------- host prep --

def _weight_fold(W, a):
    """Wf[k, h] = sum_c W[k, h*C+c] * a[h, c]  — host-side weight transform."""
    hh, cc = a.shape
    return np.einsum("khc,hc->kh", W.reshape(W.shape[0], hh, cc), a)


def _wrap16(a):
    """Slot array [S] -> dma_gather wrapped layout [128, S//16].

    Idx i lives at [i%16, i//16], replicated into all eight 16-partition
    groups (each GpSimd Q7 core reads its own group on hardware)."""
    blk = a.reshape(-1, 16).T
    return np.tile(blk, (8, 1)).astype(a.dtype)


def _prep(inputs, use_bf16):
    tdt = ml_dtypes.bfloat16 if use_bf16 else np.float32
    ei = np.asarray(inputs["edge_index"]).astype(np.int64)
    src, dst = ei[0], ei[1]
    E = src.shape[0]
    ea = np.asarray(inputs["edge_attr"]).astype(np.float32)
    x = np.asarray(inputs["x"]).astype(np.float32)

    order = np.argsort(dst, kind="stable")

    w = {k: np.asarray(v).astype(np.float32) for k, v in inputs.items()
         if k not in ("x", "edge_index", "edge_attr")}
    m3 = np.concatenate([
        _weight_fold(w["we0"], w["ae0"]),
        _weight_fold(w["we1"], w["ae1"]),
        np.pad(_weight_fold(w["we2"], w["ae2"]), ((0, 0), (0, 7)))],
        axis=1)  # [3, 24]

    waug1 = np.concatenate(
        [w["w1"], _weight_fold(w["w1"], w["as1"]),
         _weight_fold(w["w1"], w["ad1"])], axis=1)  # [512, 528]
    waug2 = np.concatenate(
        [w["w2"], _weight_fold(w["w2"], w["as2"]),
         _weight_fold(w["w2"], w["ad2"])], axis=1)  # [512, 66]
    waug0s = np.concatenate(
        [_weight_fold(w["w0"], w["as0"]), _weight_fold(w["w0"], w["ad0"])],
        axis=1)  # [64, 16]

    shared = {
        "m3": m3.astype(np.float32),
        "m3rep": np.tile(m3.reshape(1, 72), (128, 1)).astype(np.float32),
        "encw1": w["enc_w1"],
        "encb1": w["enc_b1"].reshape(ENC, 1),
        "encw2": w["enc_w2"].astype(tdt),
        "encb2": w["enc_b2"].reshape(C, 1),
        "waug0s": waug0s.astype(tdt),
        "w0": w["w0"].astype(tdt),
        "waug1": waug1[:, 0:HC].reshape(4, 128, HC).astype(tdt),
        "waug1s": waug1[:, HC:HC + 16].reshape(4, 128, 16).astype(tdt),
        "waug2": waug2.reshape(4, 128, C + 2).astype(tdt),
        "b0r": np.tile(w["b0"].reshape(1, HC), (128, 1)).astype(np.float32),
        "b1r": np.tile(w["b1"].reshape(1, HC), (128, 1)).astype(np.float32),
        "b2r": np.tile(w["b2"].reshape(1, C), (128, 1)).astype(np.float32),
        "mw1s": w["mw1"][0:C].astype(ml_dtypes.bfloat16),
        "mw1d": w["mw1"][C:2 * C].astype(ml_dtypes.bfloat16),
        "mw2": w["mw2"].astype(ml_dtypes.bfloat16),
        "mb1": w["mb1"].reshape(C, 1).astype(np.float32),
        "mb2": w["mb2"].reshape(3, 1).astype(np.float32),
        "iotaf": np.tile(np.arange(128, dtype=np.int16)[None, :], (128, 1)),
        "identf": np.eye(128, dtype=np.float32).astype(tdt),
    }

    in_maps = []
    slot_edge_ids = []
    xT = x.T.copy()  # [2, N]
    for k in range(NCORES):
        n0 = k * NPC
        sel = order[(dst[order] >= n0) & (dst[order] < n0 + NPC)]
        src_k, dst_k = src[sel], dst[sel]

        src_slot = np.zeros(S, np.int64)
        dstloc = np.full(S, -1, np.int16)
        td_slot = np.zeros(S, np.int64)
        md_slot = np.zeros(S, np.int64)
        ea_slot = np.zeros((S, EDIM), np.float32)
        pv_slot = np.zeros((S, 4), np.float32)
        eid_slot = np.full(S, -1, np.int64)

        for b in range(NB):
            nb0 = n0 + b * 128
            nreal = min(128, NPC - b * 128)
            m = (dst_k >= nb0) & (dst_k < nb0 + 128)
            idxs = np.nonzero(m)[0]
            cnt = len(idxs)
            if cnt > ECPB * 128:
                raise OverflowError(
                    f"core {k} block {b} has {cnt} edges > {ECPB * 128}")
            base = b * SPB
            sl = slice(base, base + cnt)
            src_slot[sl] = src_k[idxs]
            dstloc[sl] = (dst_k[idxs] - nb0).astype(np.int16)
            td_slot[sl] = dst_k[idxs] - n0
            md_slot[sl] = dst_k[idxs]
            ea_slot[sl] = ea[sel[idxs]]
            pv_slot[sl, 0:3] = ea[sel[idxs]]
            pv_slot[sl, 3] = 1.0
            eid_slot[sl] = sel[idxs]
            # self-loop chunk (last chunk of the block)
            lbase = base + ECPB * 128
            nodes = np.arange(nreal)
            src_slot[lbase:lbase + nreal] = nb0 + nodes
            dstloc[lbase:lbase + nreal] = nodes.astype(np.int16)
            td_slot[lbase:lbase + nreal] = b * 128 + nodes
            md_slot[lbase:lbase + nreal] = nb0 + nodes

        xTk = np.zeros((2, NB * 128), np.float32)
        xTk[:, :NPC] = xT[:, n0:n0 + NPC]
        im = dict(shared)
        im.update({
            "xT": xTk,
            "srcidx": _wrap16(src_slot.astype(np.int16)),
            "tdidx": _wrap16(td_slot.astype(np.int16)),
            "mdidx": _wrap16(md_slot.astype(np.int16)),
            "dstloc": dstloc.reshape(CH, 128).T.copy(),
            "prevals": pv_slot.reshape(CH, 128, 4).transpose(1, 0, 2)
                       .reshape(128, CH * 4).astype(tdt),
            "eaT": ea_slot.T.copy(),
        })
        in_maps.append(im)
        slot_edge_ids.append(eid_slot)
    return in_maps, slot_edge_ids, E


# ---------------------------------------------------------------- runner --

def _make_runner(nc):
    import jax
    from jax.sharding import Mesh, PartitionSpec
    from jax.experimental.shard_map import shard_map
    import concourse.mybir as mybir
    from concourse.bass2jax import (_bass_exec_p, install_neuronx_cc_hook,
                                    partition_id_tensor)

    install_neuronx_cc_hook()
    partition_name = (nc.partition_id_tensor.name
                      if nc.partition_id_tensor else None)
    in_names, out_names, out_avals, zero_outs = [], [], [], []
    for alloc in nc.m.functions[0].allocations:
        if not isinstance(alloc, mybir.MemoryLocationSet):
            continue
        name = alloc.memorylocations[0].name
        if alloc.kind == "ExternalInput":
            if name != partition_name:
                in_names.append(name)
        elif alloc.kind == "ExternalOutput":
            shape = tuple(alloc.tensor_shape)
            dtype = mybir.dt.np(alloc.dtype)
            out_names.append(name)
            out_avals.append(jax.core.ShapedArray(shape, dtype))
            zero_outs.append(np.zeros(shape, dtype))
    n_params = len(in_names)
    all_in = list(in_names) + list(out_names)
    if partition_name is not None:
        all_in.append(partition_name)

    def _body(*args):
        operands = list(args)
        if partition_name is not None:
            operands.append(partition_id_tensor())
        outs = _bass_exec_p.bind(
            *operands, out_avals=tuple(out_avals), in_names=tuple(all_in),
            out_names=tuple(out_names), lowering_input_output_aliases=(),
            sim_require_finite=False, sim_require_nnan=False, nc=nc)
        return tuple(outs)

    devices = jax.devices()[:NCORES]
    mesh = Mesh(np.asarray(devices), ("core",))
    specs = (PartitionSpec("core"),) * (n_params + len(out_names))
    sharded = jax.jit(
        shard_map(_body, mesh=mesh, in_specs=specs,
                  out_specs=(PartitionSpec("core"),) * len(out_names),
                  check_rep=False),
        keep_unused=True)
    concat_zeros = [np.zeros((NCORES * z.shape[0], *z.shape[1:]), z.dtype)
                    for z in zero_outs]

    def run(in_maps):
        import jax as _j
        concat_in = [
            np.concatenate([np.asarray(in_maps[c][nm]) for c in range(NCORES)],
                           axis=0)
            for nm in in_names]
        out_arrs = sharded(*concat_in, *concat_zeros)
        _j.block_until_ready(out_arrs)
        return [
            {nm: np.asarray(out_arrs[i]).reshape(NCORES, *out_avals[i].shape)[c]
             for i, nm in enumerate(out_names)}
            for c in range(NCORES)]

    return run


_RUNNER = None


def _get_runner(use_bf16, stages=99):
    global _RUNNER
    if _RUNNER is None:
        nc = _build_program(use_bf16, stages)
        nc.compile()
        _RUNNER = _make_runner(nc)
    return _RUNNER


def kernel(**inputs):
    in_maps, slot_edge_ids, E = _prep(inputs, USE_BF16)
    run = _get_runner(USE_BF16)
    results = run(in_maps)
    out = np.zeros((E, 3), np.float32)
    for k in range(NCORES):
        eids = slot_edge_ids[k]
        m = eids >= 0
        out[eids[m]] = results[k]["out"].T[m]
    return out

